# revision 27
# baseline (speedup 1.0000x reference)
"""LoRA linear kernel for Trainium2 (8 NeuronCores, SPMD data-parallel).

Computes y = x @ (B @ A)^T for
    x: [4, 2048, 4096] f32, B: [4096, 16] f32, A: [16, 4096] f32.

Strategy: never materialize W = B @ A.  Factor as t = x @ A^T (rank 16)
then y = t @ B^T.  Tokens (4*2048 = 8192) are sharded across 8 cores
(1024 tokens each); A and B are replicated.

The kernel is HBM-bandwidth bound (~358 GB/s/core), so x is staged and
y is returned in float16 (tolerance is 2e-2; fp16 end-to-end gives
~7e-4), halving HBM traffic vs fp32.

Pipeline: 8 independent chunks of 128 tokens.  Per chunk (1 MiB in,
1 MiB out) the dataflow is x-DMA -> mm1 -> cast -> mm2 -> cast -> y-DMA
with a few us of latency, so y writes start streaming while x is still
loading and the HBM bus never idles.

mm1 is 2-way column-tiled: two concurrent rank-16 matmuls in 32-column
strips of the PE array (tile_position), accumulating ko = j (mod 2)
partials into the 16-row bands at PSUM rows {0..15, 32..47}.  Each
chunk's mm1 opens with a dummy zero matmul covering rows 0..63: it
clears has_written for the bank, zero-fills the gap rows, and its
write-after-write overlap with both bands forces the scheduler to
order it first, making the 32 accumulating matmuls (all start=False)
order-independent.  mm2 contracts K=64 against bt64 = B^T at partition
rows {0..15, 32..47} with ZEROS elsewhere: the two bands sum into y
and the zero rows null the (zeroed) gap partitions.  PSUM->SBUF casts
are partition-preserving and split across DVE and ACT.

DMA-descriptor discipline (descriptor structure follows the SBUF
tile's innermost contiguous run): x chunks are flat [128, 8 KiB] tiles
-> 8 KiB per-partition descriptors (line rate); y goes out as two
[128, 4 KiB] half-rows per chunk so the first half streams while the
second half's copies run.  x owns the Sync HWDGE ring; bt + y own the
GpSimd SWDGE ring (an ACT-issued y DMA would stall ACT's own copies in
its FIFO queue, and on the Sync ring it would queue behind 8 MiB of x
descriptors).
"""

import sys

import numpy as np

if "/opt/trn_rl_repo" not in sys.path:
    sys.path.insert(0, "/opt/trn_rl_repo")

# Problem shape (hardcoded per contract)
BATCH = 4
SEQ = 2048
D = 4096          # in_features == out_features
R = 16            # lora rank
NCORES = 8
NTOK = BATCH * SEQ            # 8192 tokens total
TOK = NTOK // NCORES          # 1024 tokens per core
P = 128                       # partitions
KO = D // P                   # 32 feature chunks
TC = 128                      # tokens per pipeline chunk
NCH = TOK // TC               # 8 chunks per core
NB = 512                      # mm2 free dim (psum bank: 512 fp32)
KB = 64                       # mm2 contraction (2 bands of 16 + zero gaps)

# Module-level knobs for test.py (harness never touches these)
TRACE = False
LAST_RESULTS = None

_nc_cache = None


def _build_program():
    from concourse import bacc, mybir, tile

    # Bacc (not raw Bass): its finalize() runs generate_event_semaphores,
    # which splits multi-sem waits to satisfy TRN2's 1-wait-per-instruction
    # hardware constraint (walrus rejects >1 otherwise).
    nc = bacc.Bacc(
        "TRN2", target_bir_lowering=False, debug=False, num_devices=NCORES
    )

    f32 = mybir.dt.float32
    f16 = mybir.dt.float16

    xt = nc.dram_tensor("xt", [NCH, P, KO * TC], f16, kind="ExternalInput")
    # at carries a trailing [P, KB] ZERO block: the stationary operand of
    # each chunk's bank-clearing dummy matmul.
    at = nc.dram_tensor("at", [P, KO * R + KB], f16, kind="ExternalInput")
    bt = nc.dram_tensor("bt", [KB, D], f16, kind="ExternalInput")
    y = nc.dram_tensor("y", [TOK, D], f16, kind="ExternalOutput")

    with tile.TileContext(nc) as tc:
        with (
            tc.tile_pool(name="consts", bufs=1) as consts,
            tc.tile_pool(name="xin", bufs=NCH) as xin,
            tc.tile_pool(name="tbuf", bufs=2) as tbuf,
            tc.tile_pool(name="yout", bufs=6) as yout,
            tc.tile_pool(name="pt", bufs=2, space="PSUM") as pt_pool,
            tc.tile_pool(name="py", bufs=3, space="PSUM") as py_pool,
        ):
            at_s = consts.tile([P, KO * R + KB], f16)
            nc.sync.dma_start(at_s[:], at[:])
            bt_s = consts.tile([KB, D], f16)
            nc.gpsimd.dma_start(bt_s[:], bt[:])

            # Warm-up matmuls (at-based, so they only gate on the tiny at
            # DMA): keep PE streaming during the x prologue so the HAM
            # clock gate reaches K=8/8 early.
            for _ in range(2):
                warm_t = pt_pool.tile([P, TC], f32, tag="psum_t")
                nc.tensor.matmul(warm_t[:], at_s[:, :P], at_s[:, :TC], start=True, stop=True)
            for _ in range(2):
                warm = py_pool.tile([P, 2, NB], f32, tag="psum_y")
                nc.tensor.matmul(warm[:, 0, :], at_s[:, :P], at_s[:, :NB], start=True, stop=True)
            tc.no_sync_barrier()

            def mm1_chunk(xt_tile, psum_t):
                # Bank-clearing dummy: zero lhsT, [64, TC] output covering
                # both bands and their gaps.
                nc.tensor.matmul(
                    psum_t[:KB, :],
                    at_s[:, KO * R : KO * R + KB],
                    xt_tile[:, :TC],
                    start=True,
                    stop=False,
                    skip_group_check=True,
                )
                # 2-way column tiling: two concurrent rank-16 matmuls;
                # band j (psum rows 32j..32j+16) accumulates ko = j (mod 2).
                for g in range(KO // 2):
                    for j in range(2):
                        ko = g * 2 + j
                        nc.tensor.matmul(
                            psum_t[32 * j : 32 * j + R, :],
                            at_s[:, ko * R : (ko + 1) * R],
                            xt_tile[:, ko * TC : (ko + 1) * TC],
                            start=False,
                            stop=(ko == KO - 1),
                            tile_position=(0, 32 * j),
                            skip_group_check=True,
                        )

            def round_t(psum_t):
                # fp32 PSUM -> fp16 SBUF, partition-preserving (bt64's
                # zero rows null the zeroed gap partitions in mm2).
                tT = tbuf.tile([KB, TC], f16)
                nc.vector.tensor_copy(tT[: KB // 2, :], psum_t[: KB // 2, :])
                nc.scalar.copy(tT[KB // 2 :, :], psum_t[KB // 2 : KB, :])
                return tT

            def mm2_chunk(c, tT):
                y_row = yout.tile([P, D], f16)
                for pair in range(D // (2 * NB)):
                    # Two n-slices into one 2-bank PSUM tile, evacuated by
                    # a single [128, 1024] fp32->fp16 copy.
                    psum_y = py_pool.tile([P, 2, NB], f32, tag="psum_y")
                    for k in range(2):
                        n = 2 * pair + k
                        nc.tensor.matmul(
                            psum_y[:, k, :],
                            tT[:],
                            bt_s[:, n * NB : (n + 1) * NB],
                            start=True,
                            stop=True,
                        )
                    n0 = 2 * pair * NB
                    if pair % 2 == 1:
                        nc.scalar.copy(y_row[:, n0 : n0 + 2 * NB], psum_y[:])
                    else:
                        nc.vector.tensor_copy(y_row[:, n0 : n0 + 2 * NB], psum_y[:])
                    if pair == 1:
                        nc.gpsimd.dma_start(
                            y[c * TC : (c + 1) * TC, : D // 2],
                            y_row[:, : D // 2],
                        )
                nc.gpsimd.dma_start(
                    y[c * TC : (c + 1) * TC, D // 2 :], y_row[:, D // 2 :]
                )

            # Uniform 128-token pipeline; emission order == x arrival order
            # (PE is FIFO: a matmul waiting on a late DMA blocks everything
            # behind it).
            for c in range(NCH):
                xt_tile = xin.tile([P, KO * TC], f16, tag="xt")
                nc.sync.dma_start(xt_tile[:], xt[c])
                psum_t = pt_pool.tile([P, TC], f32, tag="psum_t")
                mm1_chunk(xt_tile, psum_t)
                tT = round_t(psum_t)
                mm2_chunk(c, tT)

    nc.finalize()
    return nc


def kernel(x, lora_matrix_B, lora_matrix_A):
    global _nc_cache, LAST_RESULTS
    from concourse.bass_utils import run_bass_kernel_spmd

    if _nc_cache is None:
        _nc_cache = _build_program()
    nc = _nc_cache

    x_flat = np.asarray(x, dtype=np.float32).reshape(NTOK, D).astype(np.float16)
    A = np.asarray(lora_matrix_A, dtype=np.float32).astype(np.float16)
    B = np.asarray(lora_matrix_B, dtype=np.float32).astype(np.float16)

    # at[p, ko*R + j] = A[j, ko*128 + p], then a [P, KB] zero block;
    # bt[32g + j, o] = B[o, j] for g in {0, 1}, 0 in gaps
    at_prep = np.zeros((P, KO * R + KB), dtype=np.float16)
    at_prep[:, : KO * R] = A.reshape(R, KO, P).transpose(2, 1, 0).reshape(P, KO * R)
    bt_prep = np.zeros((KB, D), dtype=np.float16)
    for g in range(2):
        bt_prep[32 * g : 32 * g + R, :] = B.T

    in_maps = []
    for c in range(NCORES):
        xc = x_flat[c * TOK : (c + 1) * TOK, :]
        # xt[ch, p, ko*TC + t] = xc[ch*TC + t, ko*128 + p]
        xt_prep = np.ascontiguousarray(
            xc.reshape(NCH, TC, KO, P).transpose(0, 3, 2, 1)
        ).reshape(NCH, P, KO * TC)
        in_maps.append({"xt": xt_prep, "at": at_prep, "bt": bt_prep})

    res = run_bass_kernel_spmd(
        nc, in_maps, core_ids=list(range(NCORES)), trace=TRACE
    )
    LAST_RESULTS = res

    y = np.concatenate(
        [np.asarray(res.results[c]["y"]) for c in range(NCORES)], axis=0
    )
    return y.reshape(BATCH, SEQ, D).astype(np.float32)


# revision 28
# speedup vs baseline: 1.3327x; 1.3327x over previous
"""LoRA linear kernel for Trainium2 (8 NeuronCores, SPMD data-parallel).

Computes y = x @ (B @ A)^T for
    x: [4, 2048, 4096] f32, B: [4096, 16] f32, A: [16, 4096] f32.

Strategy: never materialize W = B @ A.  Factor as t = x @ A^T (rank 16)
then y = t @ B^T.  Tokens (4*2048 = 8192) are sharded across 8 cores
(1024 tokens each); A and B are replicated.

The kernel is HBM-bandwidth bound (~358 GB/s/core), so x is staged and
y is returned in float16 (tolerance is 2e-2; fp16 end-to-end gives
~7e-4), halving HBM traffic vs fp32.

Pipeline: 8 independent chunks of 128 tokens.  Per chunk (1 MiB in,
1 MiB out) the dataflow is x-DMA -> mm1 -> cast -> mm2 -> cast -> y-DMA
with a few us of latency, so y writes start streaming while x is still
loading and the HBM bus never idles.

mm1 is 2-way column-tiled: two concurrent rank-16 matmuls in 32-column
strips of the PE array (tile_position), accumulating ko = j (mod 2)
partials into the 16-row bands at PSUM rows {0..15, 32..47}.  Each
chunk's mm1 opens with a dummy zero matmul covering rows 0..63: it
clears has_written for the bank, zero-fills the gap rows, and its
write-after-write overlap with both bands forces the scheduler to
order it first, making the 32 accumulating matmuls (all start=False)
order-independent.  mm2 contracts K=64 against bt64 = B^T at partition
rows {0..15, 32..47} with ZEROS elsewhere: the two bands sum into y
and the zero rows null the (zeroed) gap partitions.  PSUM->SBUF casts
are partition-preserving and split across DVE and ACT.

DMA-descriptor discipline (descriptor structure follows the SBUF
tile's innermost contiguous run): x chunks are flat [128, 8 KiB] tiles
-> 8 KiB per-partition descriptors (line rate); y goes out as two
[128, 4 KiB] half-rows per chunk so the first half streams while the
second half's copies run.  x owns the Sync HWDGE ring; bt + y own the
GpSimd SWDGE ring (an ACT-issued y DMA would stall ACT's own copies in
its FIFO queue, and on the Sync ring it would queue behind 8 MiB of x
descriptors).
"""

import sys

import numpy as np

if "/opt/trn_rl_repo" not in sys.path:
    sys.path.insert(0, "/opt/trn_rl_repo")

# Problem shape (hardcoded per contract)
BATCH = 4
SEQ = 2048
D = 4096          # in_features == out_features
R = 16            # lora rank
NCORES = 8
NTOK = BATCH * SEQ            # 8192 tokens total
TOK = NTOK // NCORES          # 1024 tokens per core
P = 128                       # partitions
KO = D // P                   # 32 feature chunks
TC = 128                      # tokens per pipeline chunk
NCH = TOK // TC               # 8 chunks per core
NB = 512                      # mm2 free dim (psum bank: 512 fp32)
KB = 128                      # mm2 contraction (4 bands of 16 + zero gaps)

# Module-level knobs for test.py (harness never touches these)
TRACE = False
LAST_RESULTS = None

_nc_cache = None


def _build_program():
    from concourse import bacc, mybir, tile

    # Bacc (not raw Bass): its finalize() runs generate_event_semaphores,
    # which splits multi-sem waits to satisfy TRN2's 1-wait-per-instruction
    # hardware constraint (walrus rejects >1 otherwise).
    nc = bacc.Bacc(
        "TRN2", target_bir_lowering=False, debug=False, num_devices=NCORES
    )

    f32 = mybir.dt.float32
    f16 = mybir.dt.float16

    xt = nc.dram_tensor("xt", [NCH, P, KO * TC], f16, kind="ExternalInput")
    # at carries a trailing [P, KB] ZERO block: the stationary operand of
    # each chunk's bank-clearing dummy matmul.
    at = nc.dram_tensor("at", [P, KO * R + KB], f16, kind="ExternalInput")
    bt = nc.dram_tensor("bt", [KB, D], f16, kind="ExternalInput")
    y = nc.dram_tensor("y", [TOK, D], f16, kind="ExternalOutput")

    with tile.TileContext(nc) as tc:
        with (
            tc.tile_pool(name="consts", bufs=1) as consts,
            tc.tile_pool(name="xin", bufs=NCH) as xin,
            tc.tile_pool(name="tbuf", bufs=2) as tbuf,
            tc.tile_pool(name="yout", bufs=6) as yout,
            tc.tile_pool(name="pt", bufs=2, space="PSUM") as pt_pool,
            tc.tile_pool(name="py", bufs=3, space="PSUM") as py_pool,
        ):
            at_s = consts.tile([P, KO * R + KB], f16)
            nc.sync.dma_start(at_s[:], at[:])
            bt_s = consts.tile([KB, D], f16)
            nc.gpsimd.dma_start(bt_s[:], bt[:])

            # Warm-up matmuls (at-based, so they only gate on the tiny at
            # DMA): keep PE streaming during the x prologue so the HAM
            # clock gate reaches K=8/8 early.
            for _ in range(2):
                warm_t = pt_pool.tile([P, TC], f32, tag="psum_t")
                nc.tensor.matmul(warm_t[:], at_s[:, :P], at_s[:, :TC], start=True, stop=True)
            for _ in range(2):
                warm = py_pool.tile([P, 2, NB], f32, tag="psum_y")
                nc.tensor.matmul(warm[:, 0, :], at_s[:, :P], at_s[:, :NB], start=True, stop=True)
            tc.no_sync_barrier()

            def mm1_chunk(xt_tile, psum_t):
                # Bank-clearing dummy: zero lhsT, [64, TC] output covering
                # both bands and their gaps.
                nc.tensor.matmul(
                    psum_t[:KB, :],
                    at_s[:, KO * R : KO * R + KB],
                    xt_tile[:, :TC],
                    start=True,
                    stop=False,
                    skip_group_check=True,
                )
                # 4-way column tiling: four concurrent rank-16 matmuls;
                # band j (psum rows 32j..32j+16) accumulates ko = j (mod 4).
                for g in range(KO // 4):
                    for j in range(4):
                        ko = g * 4 + j
                        nc.tensor.matmul(
                            psum_t[32 * j : 32 * j + R, :],
                            at_s[:, ko * R : (ko + 1) * R],
                            xt_tile[:, ko * TC : (ko + 1) * TC],
                            start=False,
                            stop=(ko == KO - 1),
                            tile_position=(0, 32 * j),
                            skip_group_check=True,
                        )

            def round_t(psum_t):
                # fp32 PSUM -> fp16 SBUF, partition-preserving (bt64's
                # zero rows null the zeroed gap partitions in mm2).
                tT = tbuf.tile([KB, TC], f16)
                nc.vector.tensor_copy(tT[: KB // 2, :], psum_t[: KB // 2, :])
                nc.scalar.copy(tT[KB // 2 :, :], psum_t[KB // 2 :, :])
                return tT

            def mm2_chunk(c, tT):
                y_row = yout.tile([P, D], f16)
                for pair in range(D // (2 * NB)):
                    # Two n-slices into one 2-bank PSUM tile, evacuated by
                    # a single [128, 1024] fp32->fp16 copy.
                    psum_y = py_pool.tile([P, 2, NB], f32, tag="psum_y")
                    for k in range(2):
                        n = 2 * pair + k
                        nc.tensor.matmul(
                            psum_y[:, k, :],
                            tT[:],
                            bt_s[:, n * NB : (n + 1) * NB],
                            start=True,
                            stop=True,
                        )
                    n0 = 2 * pair * NB
                    if pair % 2 == 1:
                        nc.scalar.copy(y_row[:, n0 : n0 + 2 * NB], psum_y[:])
                    else:
                        nc.vector.tensor_copy(y_row[:, n0 : n0 + 2 * NB], psum_y[:])
                nc.gpsimd.dma_start(y[c * TC : (c + 1) * TC, :], y_row[:])

            # Uniform 128-token pipeline; emission order == x arrival order
            # (PE is FIFO: a matmul waiting on a late DMA blocks everything
            # behind it).
            for c in range(NCH):
                xt_tile = xin.tile([P, KO * TC], f16, tag="xt")
                nc.sync.dma_start(xt_tile[:], xt[c])
                psum_t = pt_pool.tile([P, TC], f32, tag="psum_t")
                mm1_chunk(xt_tile, psum_t)
                tT = round_t(psum_t)
                mm2_chunk(c, tT)

    nc.finalize()
    return nc


def kernel(x, lora_matrix_B, lora_matrix_A):
    global _nc_cache, LAST_RESULTS
    from concourse.bass_utils import run_bass_kernel_spmd

    if _nc_cache is None:
        _nc_cache = _build_program()
    nc = _nc_cache

    x_flat = np.asarray(x, dtype=np.float32).reshape(NTOK, D).astype(np.float16)
    A = np.asarray(lora_matrix_A, dtype=np.float32).astype(np.float16)
    B = np.asarray(lora_matrix_B, dtype=np.float32).astype(np.float16)

    # at[p, ko*R + j] = A[j, ko*128 + p], then a [P, KB] zero block;
    # bt[32g + j, o] = B[o, j] for g in {0, 1}, 0 in gaps
    at_prep = np.zeros((P, KO * R + KB), dtype=np.float16)
    at_prep[:, : KO * R] = A.reshape(R, KO, P).transpose(2, 1, 0).reshape(P, KO * R)
    bt_prep = np.zeros((KB, D), dtype=np.float16)
    for g in range(4):
        bt_prep[32 * g : 32 * g + R, :] = B.T

    in_maps = []
    for c in range(NCORES):
        xc = x_flat[c * TOK : (c + 1) * TOK, :]
        # xt[ch, p, ko*TC + t] = xc[ch*TC + t, ko*128 + p]
        xt_prep = np.ascontiguousarray(
            xc.reshape(NCH, TC, KO, P).transpose(0, 3, 2, 1)
        ).reshape(NCH, P, KO * TC)
        in_maps.append({"xt": xt_prep, "at": at_prep, "bt": bt_prep})

    res = run_bass_kernel_spmd(
        nc, in_maps, core_ids=list(range(NCORES)), trace=TRACE
    )
    LAST_RESULTS = res

    y = np.concatenate(
        [np.asarray(res.results[c]["y"]) for c in range(NCORES)], axis=0
    )
    return y.reshape(BATCH, SEQ, D).astype(np.float32)


# revision 32
# speedup vs baseline: 1.4478x; 1.0864x over previous
"""LoRA linear kernel for Trainium2 (8 NeuronCores, SPMD data-parallel).

Computes y = x @ (B @ A)^T for
    x: [4, 2048, 4096] f32, B: [4096, 16] f32, A: [16, 4096] f32.

Strategy: never materialize W = B @ A.  Factor as t = x @ A^T (rank 16)
then y = t @ B^T.  Tokens (4*2048 = 8192) are sharded across 8 cores
(1024 tokens each); A and B are replicated.

The kernel is HBM-bandwidth bound (~358 GB/s/core), so x is staged and
y is returned in float16 (tolerance is 2e-2; fp16 end-to-end gives
~7e-4), halving HBM traffic vs fp32.

Pipeline: 8 independent chunks of 128 tokens.  Per chunk (1 MiB in,
1 MiB out) the dataflow is x-DMA -> mm1 -> cast -> mm2 -> cast -> y-DMA
with a few us of latency, so y writes start streaming while x is still
loading and the HBM bus never idles.

mm1 is 4-way column-tiled: four concurrent rank-16 matmuls in the
32-column strips of the PE array (tile_position), accumulating
ko = j (mod 4) partials into the 16-row bands at PSUM rows 32j..32j+15.
Each chunk's mm1 opens with a dummy zero matmul covering all 128 rows:
it clears has_written for the bank, zero-fills the gap rows, and its
write-after-write overlap with every band forces the scheduler to
order it first, making the 32 accumulating matmuls (all start=False)
order-independent.  mm2 contracts K=128 against bt128 = B^T at
partition rows {32g..32g+15} with ZEROS elsewhere: the four bands sum
into y and the zero rows null the (zeroed) gap partitions.  PSUM->SBUF
casts are partition-preserving and split across DVE and ACT; mm2
output pairs share a 2-bank PSUM tile so each [128, 1024] fp32->fp16
copy amortizes the fixed per-op PSUM-read cost.

DMA-descriptor discipline (descriptor structure follows the SBUF
tile's innermost contiguous run): x chunks and y rows are flat
[128, 8 KiB] tiles -> 8 KiB per-partition descriptors (line rate).
at + bt + x own the Sync HWDGE ring; y owns the GpSimd SWDGE ring (an
ACT-issued y DMA would stall ACT's own copies in its FIFO queue, and
on the Sync ring it would queue behind 8 MiB of x descriptors).
"""

import sys

import numpy as np

if "/opt/trn_rl_repo" not in sys.path:
    sys.path.insert(0, "/opt/trn_rl_repo")

# Problem shape (hardcoded per contract)
BATCH = 4
SEQ = 2048
D = 4096          # in_features == out_features
R = 16            # lora rank
NCORES = 8
NTOK = BATCH * SEQ            # 8192 tokens total
TOK = NTOK // NCORES          # 1024 tokens per core
P = 128                       # partitions
KO = D // P                   # 32 feature chunks
TC = 128                      # tokens per pipeline chunk
NCH = TOK // TC               # 8 chunks per core
NB = 512                      # mm2 free dim (psum bank: 512 fp32)
KB = 128                      # mm2 contraction (4 bands of 16 + zero gaps)

# Module-level knobs for test.py (harness never touches these)
TRACE = False
LAST_RESULTS = None

_nc_cache = None


def _build_program():
    from concourse import bacc, mybir, tile

    # Bacc (not raw Bass): its finalize() runs generate_event_semaphores,
    # which splits multi-sem waits to satisfy TRN2's 1-wait-per-instruction
    # hardware constraint (walrus rejects >1 otherwise).
    nc = bacc.Bacc(
        "TRN2", target_bir_lowering=False, debug=False, num_devices=NCORES
    )

    f32 = mybir.dt.float32
    f16 = mybir.dt.float16

    xt = nc.dram_tensor("xt", [NCH, P, KO * TC], f16, kind="ExternalInput")
    # at carries a trailing [P, KB] ZERO block: the stationary operand of
    # each chunk's bank-clearing dummy matmul.
    at = nc.dram_tensor("at", [P, KO * R + KB], f16, kind="ExternalInput")
    bt = nc.dram_tensor("bt", [KB, D], f16, kind="ExternalInput")
    y = nc.dram_tensor("y", [TOK, D], f16, kind="ExternalOutput")

    with tile.TileContext(nc) as tc:
        with (
            tc.tile_pool(name="consts", bufs=1) as consts,
            tc.tile_pool(name="xin", bufs=NCH) as xin,
            tc.tile_pool(name="tbuf", bufs=2) as tbuf,
            tc.tile_pool(name="yout", bufs=6) as yout,
            tc.tile_pool(name="pt", bufs=2, space="PSUM") as pt_pool,
            tc.tile_pool(name="py", bufs=3, space="PSUM") as py_pool,
        ):
            at_s = consts.tile([P, KO * R + KB], f16)
            nc.sync.dma_start(at_s[:], at[:])
            bt_s = consts.tile([KB, D], f16)
            nc.sync.dma_start(bt_s[:], bt[:])

            # Warm-up matmuls (at-based, so they only gate on the tiny at
            # DMA): keep PE streaming during the x prologue so the HAM
            # clock gate reaches K=8/8 early.
            for _ in range(2):
                warm_t = pt_pool.tile([P, TC], f32, tag="psum_t")
                nc.tensor.matmul(warm_t[:], at_s[:, :P], at_s[:, :TC], start=True, stop=True)
            for _ in range(2):
                warm = py_pool.tile([P, 2, NB], f32, tag="psum_y")
                nc.tensor.matmul(warm[:, 0, :], at_s[:, :P], at_s[:, :NB], start=True, stop=True)
            tc.no_sync_barrier()

            def mm1_chunk(xt_tile, psum_t):
                # Bank-clearing dummy: zero lhsT, [128, TC] output covering
                # all bands and their gaps.
                nc.tensor.matmul(
                    psum_t[:KB, :],
                    at_s[:, KO * R : KO * R + KB],
                    xt_tile[:, :TC],
                    start=True,
                    stop=False,
                    skip_group_check=True,
                )
                # 4-way column tiling: four concurrent rank-16 matmuls;
                # band j (psum rows 32j..32j+16) accumulates ko = j (mod 4).
                for g in range(KO // 4):
                    for j in range(4):
                        ko = g * 4 + j
                        nc.tensor.matmul(
                            psum_t[32 * j : 32 * j + R, :],
                            at_s[:, ko * R : (ko + 1) * R],
                            xt_tile[:, ko * TC : (ko + 1) * TC],
                            start=False,
                            stop=(ko == KO - 1),
                            tile_position=(0, 32 * j),
                            skip_group_check=True,
                        )

            def round_t(psum_t):
                # fp32 PSUM -> fp16 SBUF, partition-preserving (bt128's
                # zero rows null the zeroed gap partitions in mm2).
                tT = tbuf.tile([KB, TC], f16)
                nc.vector.tensor_copy(tT[: KB // 2, :], psum_t[: KB // 2, :])
                nc.scalar.copy(tT[KB // 2 :, :], psum_t[KB // 2 :, :])
                return tT

            def mm2_chunk(c, tT):
                y_row = yout.tile([P, D], f16)
                for pair in range(D // (2 * NB)):
                    # Two n-slices into one 2-bank PSUM tile, evacuated by
                    # a single [128, 1024] fp32->fp16 copy.
                    psum_y = py_pool.tile([P, 2, NB], f32, tag="psum_y")
                    for k in range(2):
                        n = 2 * pair + k
                        nc.tensor.matmul(
                            psum_y[:, k, :],
                            tT[:],
                            bt_s[:, n * NB : (n + 1) * NB],
                            start=True,
                            stop=True,
                        )
                    n0 = 2 * pair * NB
                    if pair % 2 == 1:
                        nc.scalar.copy(y_row[:, n0 : n0 + 2 * NB], psum_y[:])
                    else:
                        nc.vector.tensor_copy(y_row[:, n0 : n0 + 2 * NB], psum_y[:])
                nc.gpsimd.dma_start(y[c * TC : (c + 1) * TC, :], y_row[:])

            # Uniform 128-token pipeline; emission order == x arrival order
            # (PE is FIFO: a matmul waiting on a late DMA blocks everything
            # behind it).
            for c in range(NCH):
                xt_tile = xin.tile([P, KO * TC], f16, tag="xt")
                nc.sync.dma_start(xt_tile[:], xt[c])
                psum_t = pt_pool.tile([P, TC], f32, tag="psum_t")
                mm1_chunk(xt_tile, psum_t)
                tT = round_t(psum_t)
                mm2_chunk(c, tT)

    nc.finalize()
    return nc


def kernel(x, lora_matrix_B, lora_matrix_A):
    global _nc_cache, LAST_RESULTS
    from concourse.bass_utils import run_bass_kernel_spmd

    if _nc_cache is None:
        _nc_cache = _build_program()
    nc = _nc_cache

    x_flat = np.asarray(x, dtype=np.float32).reshape(NTOK, D).astype(np.float16)
    A = np.asarray(lora_matrix_A, dtype=np.float32).astype(np.float16)
    B = np.asarray(lora_matrix_B, dtype=np.float32).astype(np.float16)

    # at[p, ko*R + j] = A[j, ko*128 + p], then a [P, KB] zero block;
    # bt[32g + j, o] = B[o, j] for g in {0, 1}, 0 in gaps
    at_prep = np.zeros((P, KO * R + KB), dtype=np.float16)
    at_prep[:, : KO * R] = A.reshape(R, KO, P).transpose(2, 1, 0).reshape(P, KO * R)
    bt_prep = np.zeros((KB, D), dtype=np.float16)
    for g in range(4):
        bt_prep[32 * g : 32 * g + R, :] = B.T

    in_maps = []
    for c in range(NCORES):
        xc = x_flat[c * TOK : (c + 1) * TOK, :]
        # xt[ch, p, ko*TC + t] = xc[ch*TC + t, ko*128 + p]
        xt_prep = np.ascontiguousarray(
            xc.reshape(NCH, TC, KO, P).transpose(0, 3, 2, 1)
        ).reshape(NCH, P, KO * TC)
        in_maps.append({"xt": xt_prep, "at": at_prep, "bt": bt_prep})

    res = run_bass_kernel_spmd(
        nc, in_maps, core_ids=list(range(NCORES)), trace=TRACE
    )
    LAST_RESULTS = res

    y = np.concatenate(
        [np.asarray(res.results[c]["y"]) for c in range(NCORES)], axis=0
    )
    return y.reshape(BATCH, SEQ, D).astype(np.float32)
